# revision 1
# baseline (speedup 1.0000x reference)
import numpy as np
import jax
import jax.numpy as jnp

# nn_ActorNetwork: per-subactor GRU(S=6 -> H=16) over T=128 steps + 3-layer MLP head.
# Sharding: expert-parallel over the subactor axis N=100, padded to 104 = 8 cores x 13.
N, H, S = 100, 16, 6
B, T = 128, 128
NC = 8
NP = 104
NL = NP // NC  # 13


def _forward(x, Wih, Whh, bih, bhh, W1, b1, W2, b2, W3, b3):
    # x: [B,T,NL,S]; stacked per-subactor weights [NL,...]
    xi = jnp.einsum('btns,ngs->btng', x, Wih) + bih  # [B,T,NL,3H]

    def step(hprev, xt):  # hprev [B,NL,H], xt [B,NL,3H]
        hh = jnp.einsum('bnh,ngh->bng', hprev, Whh) + bhh
        xr, xz, xn = jnp.split(xt, 3, axis=-1)
        hr, hz, hn = jnp.split(hh, 3, axis=-1)
        r = jax.nn.sigmoid(xr + hr)
        z = jax.nn.sigmoid(xz + hz)
        nn_ = jnp.tanh(xn + r * hn)
        hnew = (1.0 - z) * nn_ + z * hprev
        return hnew, hnew

    h0 = jnp.zeros((B, NL, H), dtype=x.dtype)
    _, hs = jax.lax.scan(step, h0, jnp.moveaxis(xi, 1, 0))  # [T,B,NL,H]
    hs = jnp.moveaxis(hs, 0, 1)  # [B,T,NL,H]

    y = jax.nn.relu(jnp.einsum('btnh,nkh->btnk', hs, W1) + b1)
    y = jax.nn.relu(jnp.einsum('btnh,nkh->btnk', y, W2) + b2)
    y = jax.nn.relu(jnp.einsum('btnh,nkh->btnk', y, W3) + b3)  # [B,T,NL,1]
    return y[..., 0]  # [B,T,NL]


_pmapped = None


def _get_pmapped():
    global _pmapped
    if _pmapped is None:
        _pmapped = jax.pmap(_forward)
    return _pmapped


def _pad_n(a, axis):
    pad = [(0, 0)] * a.ndim
    pad[axis] = (0, NP - N)
    return np.pad(np.asarray(a, dtype=np.float32), pad)


def kernel(x, Wih, Whh, bih, bhh, W1, b1, W2, b2, W3, b3):
    xs = _pad_n(x, 2)  # [B,T,104,S]
    xs = np.ascontiguousarray(
        np.moveaxis(xs.reshape(B, T, NC, NL, S), 2, 0)
    )  # [8,B,T,13,S]; core c owns subactors c*13 .. c*13+12

    def shard_w(a):
        a = _pad_n(a, 0)
        return a.reshape(NC, NL, *a.shape[1:])

    ws = [shard_w(a) for a in (Wih, Whh, bih, bhh, W1, b1, W2, b2, W3, b3)]

    try:
        y = _get_pmapped()(xs, *ws)  # [8,B,T,13]
        y = np.asarray(y)
    except Exception:
        # fallback: run shards sequentially on the default device
        fwd = jax.jit(_forward)
        y = np.stack([np.asarray(fwd(xs[c], *[w[c] for w in ws])) for c in range(NC)])

    y_nbt = np.moveaxis(y, 3, 1).reshape(NP, B, T)[:N]  # [100,B,T]
    # reference: transpose(y,(2,0,1,3)).reshape(-1, T, N)
    return np.ascontiguousarray(y_nbt).reshape(-1, T, N).astype(np.float32)

